# revision 29
# baseline (speedup 1.0000x reference)
"""GraphSAGE (2-layer, mean aggregation) on 8 Trainium2 NeuronCores.

Sharding: nodes partitioned by dst range across 8 cores (graph parallel).
Layer-1 edge messages x[src] are a compile-time permutation, so the host
stages them as a contiguous fp8e4m3 stream (direct DMA at full bandwidth;
the fp8 quantization adds ~3e-3 rms).  Layer-2 messages h[src] are batch-
gathered with SWDGE dma_gather (1024 indices per call — the descriptor
ring's hard limit — on 4 rotating queues) from two AllGathered node-major
h tables: piece A holds every core's L1 groups 0-4 and is AllGathered
mid-L1 (hidden under compute), piece B (groups 5-9) right after L1.  Both
tables are under 32768 rows, so int16 gather indices need no rebasing;
each edge is classed A/B by its source's L1 group.  Gathers wait for a
1-row barrier AllGather after piece B — collective completion alone races
remote stripe visibility (seen as rare NaN/garbage rows) — and are pinned
behind the collective dispatches on the Pool FIFO with no-sync deps, since
overlapping SWDGE gathers starve the collective's DMA rings ~2.5x.
Segment-sum runs as TensorE matmuls against one-hot dst-selection tiles
generated on-device by one DVE iota==dst compare per group (fp8 for L1 to
match the message dtype).  Dense SAGE transforms emit node-major blocks
directly (PE contracts channels with the node-block as the stationary
operand; bias via a rank-1 e0 x b matmul), so no transposes are needed.
Node rows use a group-major permuted layout (local node 500g+125b+p at row
500g+4p+b) so every hloc/out DMA write is contiguous; the host un-permutes.
Weights are replicated; PSUM accumulates in f32; output is f32.
"""

import ml_dtypes
import numpy as np

import concourse.bass as bass
import concourse.library_config as library_config
import concourse.mybir as mybir
import concourse.tile as tile
from concourse.bass_utils import run_bass_kernel_spmd
from concourse.library_overlay import lower_extended_insts
from concourse.tile import ScopedClock

# ---------------------------------------------------------------------------
# Workarounds for this container's walrus codegen: instructions can carry at
# most one sync-wait command ("Too many sync wait commands" otherwise), and
# Drain-based barriers reject waits entirely.
# ---------------------------------------------------------------------------


def _drain_and_barrier(self, tick_clock, wait_clock):
    nop_inst = self.nc.sync.nop(nofuse=True, hint="pre_drain_waits")
    wait_clock.add_sem_waits(
        nop_inst.ins, ScopedClock({None: tick_clock.global_clock})
    )
    si = nop_inst.ins.sync_info
    waits = list(si.on_wait) if si and si.on_wait else []
    if len(waits) > 1:
        si.on_wait = waits[:1]
        for w in waits[1:]:
            extra = self.nc.sync.nop(nofuse=True, hint="pre_drain_waits_x")
            extra.ins.sync_info = type(si)(on_wait=[w], on_update=[])
    self.nc.sync.drain()
    self.nc.all_engine_barrier(sem_only=True)
    assert self.sems is not None
    popped = self.nc._tile_sem_poison_stack.pop()
    assert popped is self._sem_poison
    self.nc.clear_and_free_semaphores(list(self.sems.allocated().values()))
    self.nc.all_engine_barrier(sem_only=True)


tile.TileContext._drain_and_barrier = _drain_and_barrier


def _split_multi_waits(nc, maxw=1):
    """Move excess sync-waits onto same-engine NOPs inserted before."""
    n = 0
    for blk in nc.m.functions[0].blocks:
        il = blk.instructions
        i = 0
        while i < len(il):
            inst = il[i]
            si = inst.sync_info
            waits = list(si.on_wait) if si and si.on_wait else []
            if len(waits) > maxw:
                si.on_wait = waits[-maxw:]
                for w in waits[:-maxw]:
                    nop = mybir.InstNoOp(
                        name=f"wsplit-{n}",
                        engine=inst.engine,
                        sync_info=mybir.SyncInfo(on_wait=[w], on_update=[]),
                    )
                    n += 1
                    il.insert(i, nop)
                    i += 1
            i += 1


# ---------------------------------------------------------------------------

N = 40000
E = 640000
C = 128          # in/hidden channels
O = 121          # out channels
NCORES = 8
NLOC = N // NCORES       # 5000 dst nodes per core
DT = 50                  # dst nodes per aggregation tile
NT = NLOC // DT          # 100 dst tiles per core
GT = 10                  # tiles per pipeline group
NG = NT // GT            # 10 groups per layer
GCOL = GT * DT           # 500 agg columns per group
DBLK = 125               # nodes per dense output block
NBLK = NLOC // DBLK      # 40 dense blocks
BPG = GCOL // DBLK       # 4 dense blocks per group
P = 128                  # edges per chunk (matmul contraction dim)
DT2 = 100                # layer-2 dst tile (coarser: less ceil padding)
NT2 = NLOC // DT2        # 50 layer-2 tiles
GT2 = GCOL // DT2        # 5 layer-2 tiles per group
GBUFS = 4                # L2 gathered-message buffers in flight
NG_A = 5                 # L1 groups in AllGather piece A (piece B = rest)
F32 = mybir.dt.float32
BF16 = mybir.dt.bfloat16
FP8 = mybir.dt.float8e4
I16 = mybir.dt.int16

_cache = {}
PHASE = 5  # 1=L1, 2=+AllGather, 3=+L2 gathers, 4=+L2 agg, 5=full


def _build(meta):
    """meta = (kc1, kp1, klo, khi, kp2) chunk structure, SPMD-identical."""
    key = (meta, PHASE)
    if key in _cache:
        return _cache[key]
    kc1, kp1, klo, khi, kp2 = meta   # klo/khi = A/B half chunk counts
    kc1 = np.array(kc1)
    klo = np.array(klo)
    khi = np.array(khi)
    kc2 = klo + khi
    coff1 = np.concatenate([[0], np.cumsum(kc1)])
    nch1 = int(coff1[-1])
    nlo = int(klo.sum())
    nhi = int(khi.sum())
    # per-group A/B chunk offsets for layer 2
    glo = np.concatenate([[0], np.cumsum(klo.reshape(NG, GT2).sum(axis=1))])
    ghi = np.concatenate([[0], np.cumsum(khi.reshape(NG, GT2).sum(axis=1))])

    nc = bass.Bass(num_swdge_queues=4)
    mstream = nc.dram_tensor("mstream", [P, nch1 * C], FP8, kind="ExternalInput")
    xT = nc.dram_tensor("xT", [C, NLOC], BF16, kind="ExternalInput")
    invc = nc.dram_tensor("invc", [P, NLOC], BF16, kind="ExternalInput")
    iota1 = nc.dram_tensor("iota1", [P, DT * kp1], BF16, kind="ExternalInput")
    iota2 = nc.dram_tensor("iota2", [P, DT2 * kp2], BF16, kind="ExternalInput")
    dval1 = nc.dram_tensor("dval1", [P, NT * kp1], BF16, kind="ExternalInput")
    dval2 = nc.dram_tensor("dval2", [P, NT2 * kp2], BF16, kind="ExternalInput")
    idxlo = nc.dram_tensor("idxlo", [P, nlo * 8], I16, kind="ExternalInput")
    idxhi = nc.dram_tensor("idxhi", [P, max(nhi, 1) * 8], I16, kind="ExternalInput")
    w1lT = nc.dram_tensor("w1lT", [C, C], BF16, kind="ExternalInput")
    w1rT = nc.dram_tensor("w1rT", [C, C], BF16, kind="ExternalInput")
    w2lT = nc.dram_tensor("w2lT", [C, O], BF16, kind="ExternalInput")
    w2rT = nc.dram_tensor("w2rT", [C, O], BF16, kind="ExternalInput")
    b1c = nc.dram_tensor("b1c", [C, 1], F32, kind="ExternalInput")
    b1row = nc.dram_tensor("b1row", [P, C], BF16, kind="ExternalInput")
    b2row = nc.dram_tensor("b2row", [P, O], BF16, kind="ExternalInput")
    out = nc.dram_tensor("out", [NLOC, O], F32, kind="ExternalOutput")

    with tile.TileContext(nc) as tc:
        with (
            tc.tile_pool(name="const", bufs=1) as cpool,
            tc.tile_pool(name="feat", bufs=1) as fpool,
            tc.tile_pool(name="slab", bufs=3) as spool1,
            tc.tile_pool(name="gbig", bufs=GBUFS) as gpool,
            tc.tile_pool(name="agg", bufs=2) as apool,
            tc.tile_pool(name="oh", bufs=2) as opool,
            tc.tile_pool(name="stg", bufs=2) as spool,
            tc.tile_pool(name="psum_a", bufs=2, space="PSUM") as pa,
            tc.tile_pool(name="psum_d", bufs=2, space="PSUM") as pd,
            tc.tile_pool(name="psum_n", bufs=3, space="PSUM") as pn,
            tc.tile_pool(name="dram", bufs=1, space="DRAM") as dpool,
        ):
            # ---- resident tiles -------------------------------------------
            xT_s = fpool.tile([C, NLOC], BF16)
            invc_s = fpool.tile([P, NLOC], BF16)
            iota1_s = fpool.tile([P, DT * kp1], BF16)
            iota2_s = fpool.tile([P, DT2 * kp2], BF16)
            dval1_s = fpool.tile([P, NT * kp1], BF16)
            dval2_s = fpool.tile([P, NT2 * kp2], BF16)
            idxlo_s = fpool.tile([P, nlo * 8], I16)
            idxhi_s = fpool.tile([P, max(nhi, 1) * 8], I16)
            hT_s = fpool.tile([C, NLOC], BF16)
            w1lT_s = cpool.tile([C, C], BF16)
            w1rT_s = cpool.tile([C, C], BF16)
            w2lT_s = cpool.tile([C, O], BF16)
            w2rT_s = cpool.tile([C, O], BF16)
            b1c_s = cpool.tile([C, 1], F32)
            b1row_s = cpool.tile([P, C], BF16)
            b2row_s = cpool.tile([P, O], BF16)
            e0_s = cpool.tile([P, DBLK], BF16)
            zer_s = cpool.tile([P, 1], F32)

            hloc = dpool.tile([NLOC, C], BF16)
            htabA = dpool.tile([NCORES * NG_A * GCOL, C], BF16, addr_space="Shared")
            htabB = dpool.tile([NCORES * (NG - NG_A) * GCOL, C], BF16, addr_space="Shared")
            agbar = dpool.tile([NCORES, C], BF16, addr_space="Shared")

            nc.sync.dma_start(out=iota1_s[:], in_=iota1[:])
            nc.sync.dma_start(out=dval1_s[:], in_=dval1[:])
            nc.sync.dma_start(out=w1lT_s[:], in_=w1lT[:])
            nc.sync.dma_start(out=w1rT_s[:], in_=w1rT[:])
            nc.sync.dma_start(out=b1c_s[:], in_=b1c[:])
            nc.sync.dma_start(out=b1row_s[:], in_=b1row[:])
            nc.gpsimd.memset(e0_s[:], 0.0)
            nc.gpsimd.memset(e0_s[0:1, :], 1.0)
            nc.gpsimd.memset(zer_s[:], 0.0)
            nc.gpsimd.load_library(library_config.mlp)

            # node rows permuted so every hloc/out DMA write is contiguous:
            # local node n = 500g + 125b + p  <->  row 500g + 4p + b
            ag_insts = []
            hloc_v = hloc[:, :].rearrange("(g p b) c -> p g b c", p=DBLK, b=BPG)
            out_v = out[:, :].rearrange("(g p b) o -> p g b o", p=DBLK, b=BPG)

            def onehot(g, iota_s, dval_s, kp, dt, gt, dtype):
                """One DVE is_equal builds the group's one-hot block:
                oh[p, t*dt*kp + d*kp + j] = (dst_off(tile t, chunk j, lane p) == d)."""
                oh = opool.tile([P, gt * dt * kp], dtype, tag="oh")
                oh4 = oh[:, :].rearrange(
                    "p (t d j) -> p t d j", t=gt, d=dt, j=kp
                )
                iota4 = (
                    iota_s[:, :]
                    .rearrange("p (d j) -> p d j", j=kp)
                    .unsqueeze(1)
                    .to_broadcast((P, gt, dt, kp))
                )
                dval4 = (
                    dval_s[:, gt * kp * g : gt * kp * (g + 1)]
                    .rearrange("p (t j) -> p t j", j=kp)
                    .unsqueeze(2)
                    .to_broadcast((P, gt, dt, kp))
                )
                nc.vector.tensor_tensor(
                    out=oh4, in0=iota4, in1=dval4, op=mybir.AluOpType.is_equal
                )
                return oh4

            def aggregate(g, oh4, lhs, kcs, dt, gt):
                """Accumulate the group's [C, 500] segment-sum and normalize."""
                ps = pa.tile([C, GCOL], F32, space="PSUM")
                for t in range(gt):
                    tt = gt * g + t
                    k_t = int(kcs[tt])
                    for k in range(k_t):
                        nc.tensor.matmul(
                            out=ps[:, t * dt : (t + 1) * dt],
                            lhsT=lhs(tt, k),
                            rhs=oh4[:, t, :, k],
                            start=(k == 0),
                            stop=(k == k_t - 1),
                        )
                agg = apool.tile([C, GCOL], BF16, tag="agg")
                cols = slice(g * GCOL, (g + 1) * GCOL)
                nc.scalar.activation(
                    agg[:], ps[:], mybir.ActivationFunctionType.Copy
                )
                nc.vector.tensor_mul(
                    out=agg[:], in0=agg[:], in1=invc_s[:, cols]
                )
                return agg

            # ---- layer 1 ---------------------------------------------------
            for g in range(NG):
                j0, j1 = int(coff1[GT * g]), int(coff1[GT * (g + 1)])
                kg = j1 - j0
                slab = spool1.tile([P, kg * C], FP8, tag="slab")
                nc.sync.dma_start(
                    out=slab[:], in_=mstream[:, j0 * C : j1 * C]
                )
                if g == 0:
                    # after group 0's slab so that DMA issues first, but
                    # before any consumer of xT/invc in program order
                    # scalar HWDGE ring: parallel with slab prefetch
                    nc.scalar.dma_start(out=xT_s[:], in_=xT[:])
                    nc.scalar.dma_start(out=invc_s[:], in_=invc[:])
                if g == 1:
                    # L2-only inputs: off the L1-critical DMA prefix
                    nc.scalar.dma_start(out=iota2_s[:], in_=iota2[:])
                    nc.scalar.dma_start(out=dval2_s[:], in_=dval2[:])
                    nc.scalar.dma_start(out=idxlo_s[:], in_=idxlo[:])
                    nc.scalar.dma_start(out=idxhi_s[:], in_=idxhi[:])
                    nc.scalar.dma_start(out=w2lT_s[:], in_=w2lT[:])
                    nc.scalar.dma_start(out=w2rT_s[:], in_=w2rT[:])
                    nc.scalar.dma_start(out=b2row_s[:], in_=b2row[:])
                oh4 = onehot(g, iota1_s, dval1_s, kp1, DT, GT, FP8)
                agg = aggregate(
                    g, oh4,
                    lambda tt, k: slab[
                        :, (int(coff1[tt]) - j0 + k) * C
                        : (int(coff1[tt]) - j0 + k + 1) * C
                    ],
                    kc1, DT, GT,
                )
                cols = slice(g * GCOL, (g + 1) * GCOL)
                # channel-major dense -> hT
                ph = pd.tile([C, GCOL], F32, space="PSUM")
                nc.tensor.matmul(
                    out=ph[:], lhsT=w1lT_s[:], rhs=agg[:],
                    start=True, stop=False,
                )
                nc.tensor.matmul(
                    out=ph[:], lhsT=w1rT_s[:], rhs=xT_s[:, cols],
                    start=False, stop=True,
                )
                nc.scalar.activation(
                    hT_s[:, cols], ph[:],
                    mybir.ActivationFunctionType.Relu, bias=b1c_s[:, :1],
                )
                # node-major dense -> h blocks -> hloc (permuted rows)
                hstg = spool.tile([DBLK, BPG * C], BF16, tag="hstg")
                for b in range(BPG):
                    nb = slice((g * BPG + b) * DBLK, (g * BPG + b + 1) * DBLK)
                    pnb = pn.tile([DBLK, C], F32, space="PSUM")
                    nc.tensor.matmul(
                        out=pnb[:], lhsT=agg[:, b * DBLK : (b + 1) * DBLK],
                        rhs=w1lT_s[:],
                        start=True, stop=False,
                    )
                    nc.tensor.matmul(
                        out=pnb[:], lhsT=xT_s[:, nb], rhs=w1rT_s[:],
                        start=False, stop=False,
                    )
                    nc.tensor.matmul(
                        out=pnb[:], lhsT=e0_s[:], rhs=b1row_s[:],
                        start=False, stop=True,
                    )
                    nc.scalar.activation(
                        hstg[:, b * C : (b + 1) * C], pnb[:],
                        mybir.ActivationFunctionType.Relu,
                        bias=zer_s[:DBLK, :1],
                    )
                nc.sync.dma_start(
                    out=hloc_v[:, g, :, :],
                    in_=hstg[:, :].rearrange("p (b c) -> p b c", c=C),
                )
                # two-piece AllGather: A (groups 0-4) overlaps L1's tail,
                # B (groups 5-9) right after it; Shared outputs take the
                # fast collective path (one writer per tensor)
                if PHASE >= 2 and g in (NG_A - 1, NG - 1):
                    piece = htabA if g < NG_A else htabB
                    lo = 0 if g < NG_A else NG_A * GCOL
                    ag_insts.append(nc.gpsimd.collective_compute(
                        "AllGather",
                        mybir.AluOpType.bypass,
                        replica_groups=[list(range(NCORES))],
                        ins=[
                            hloc[
                                lo : lo
                                + (NG_A if g < NG_A else NG - NG_A) * GCOL,
                                :,
                            ].opt()
                        ],
                        outs=[piece[:, :].opt()],
                    ))
                    if g == NG - 1:
                        # 1-row barrier AllGather: completion implies every
                        # peer's AG-B stripes drained into our htab (same
                        # CC queue + link FIFO), closing the visibility race
                        ag_insts.append(nc.gpsimd.collective_compute(
                            "AllGather",
                            mybir.AluOpType.bypass,
                            replica_groups=[list(range(NCORES))],
                            ins=[hloc[0:1, :].opt()],
                            outs=[agbar[:, :].opt()],
                        ))

            # ---- layer 2 ---------------------------------------------------
            loff_lo = np.concatenate([[0], np.cumsum(klo)])
            loff_hi = np.concatenate([[0], np.cumsum(khi)])
            _nreg = {}

            def nreg(v):
                if v not in _nreg:
                    _nreg[v] = nc.gpsimd.to_reg(v)
                return _nreg[v]

            bigs = {}
            qrr = [0]
            GMAX = 8

            first_gather = []

            def gather_half(big, kgh, base_col, idx_s, goff, tab):
                for c0 in range(0, kgh, GMAX):
                    ncall = min(GMAX, kgh - c0)
                    col = base_col + c0
                    inst = nc.gpsimd.dma_gather(
                        out_ap=big[
                            :, col * C : (col + ncall) * C
                        ].rearrange("p (k c) -> p k c", c=C),
                        in_ap=tab,
                        idxs_ap=idx_s[
                            :, (goff + c0) * 8 : (goff + c0 + ncall) * 8
                        ],
                        num_idxs=ncall * P,
                        num_idxs_reg=nreg(ncall * P),
                        elem_size=C,
                        queue_num=qrr[0] % nc.num_swdge_queues,
                    )
                    # keep gathers behind both AllGather dispatches on the
                    # Pool FIFO (Tile would otherwise hoist them); the first
                    # gather waits for AllGather-B to COMPLETE so the SWDGE
                    # engines don't contend with the collective's DMA rings
                    deps = bass.InstructionNameOrderedSet()
                    for a in ag_insts:
                        deps.add(a.ins.name)
                    inst.ins.add_nosync_dependencies_from(deps)
                    if not first_gather:
                        first_gather.append(inst)
                        bass._add_dep_helper(
                            inst.ins, ag_insts[-1].ins, sync=True,
                            reason="gathers after AG-B completes",
                        )
                    qrr[0] += 1

            def prep(g):
                """Issue group g's gather calls (A half, then B half)."""
                kglo = int(glo[g + 1] - glo[g])
                kghi = int(ghi[g + 1] - ghi[g])
                big = gpool.tile([P, (kglo + kghi) * C], BF16, tag="gbig")
                gather_half(big, kglo, 0, idxlo_s, int(glo[g]), htabA[:, :])
                gather_half(
                    big, kghi, kglo, idxhi_s, int(ghi[g]), htabB[:, :]
                )
                bigs[g] = big

            def process(g):
                kglo = int(glo[g + 1] - glo[g])
                big = bigs.pop(g)
                if PHASE < 4:
                    if g + GBUFS < NG:
                        prep(g + GBUFS)
                    return

                def lhs2(tt, k, g=g, big=big, kglo=kglo):
                    if k < klo[tt]:
                        jc = int(loff_lo[tt] - glo[g]) + k
                    else:
                        jc = (
                            kglo
                            + int(loff_hi[tt] - ghi[g])
                            + (k - int(klo[tt]))
                        )
                    return big[:, jc * C : (jc + 1) * C]

                oh4 = onehot(g, iota2_s, dval2_s, kp2, DT2, GT2, BF16)
                agg = aggregate(g, oh4, lhs2, kc2, DT2, GT2)
                if PHASE < 5:
                    if g + GBUFS < NG:
                        prep(g + GBUFS)
                    return
                # node-major dense -> out blocks (permuted rows)
                ostg = spool.tile([DBLK, BPG * O], F32, tag="ostg")
                for b in range(BPG):
                    nb = slice((g * BPG + b) * DBLK, (g * BPG + b + 1) * DBLK)
                    pnb = pn.tile([DBLK, O], F32, space="PSUM")
                    nc.tensor.matmul(
                        out=pnb[:], lhsT=agg[:, b * DBLK : (b + 1) * DBLK],
                        rhs=w2lT_s[:],
                        start=True, stop=False,
                    )
                    nc.tensor.matmul(
                        out=pnb[:], lhsT=hT_s[:, nb], rhs=w2rT_s[:],
                        start=False, stop=False,
                    )
                    nc.tensor.matmul(
                        out=pnb[:], lhsT=e0_s[:], rhs=b2row_s[:],
                        start=False, stop=True,
                    )
                    nc.scalar.activation(
                        ostg[:, b * O : (b + 1) * O], pnb[:],
                        mybir.ActivationFunctionType.Copy,
                    )
                nc.sync.dma_start(
                    out=out_v[:, g, :, :],
                    in_=ostg[:, :].rearrange("p (b o) -> p b o", o=O),
                )
                if g + GBUFS < NG:
                    prep(g + GBUFS)

            if PHASE >= 3:
                # A-half gathers of the first GBUFS groups run right after
                # AllGather-A (overlapping L1's tail and AllGather-B); their
                # B halves follow so the Pool FIFO never stalls on piece B
                for g in range(GBUFS):
                    prep(g)
                for g in range(NG):
                    process(g)

    _split_multi_waits(nc)
    lower_extended_insts(nc)
    _cache[key] = nc
    return nc


def _wrap16(vals):
    """dma_gather index layout: linear idx i at partition i%16, col i//16,
    replicated across the 8 Q7-core partition stripes."""
    v = np.asarray(vals, np.int16).reshape(-1, 16)
    return np.tile(v.T, (P // 16, 1))


def _rho_loc():
    """local node n = 500g + 125b + p  ->  hloc/out row 500g + 4p + b"""
    n = np.arange(NLOC)
    g, s = n // GCOL, n % GCOL
    return g * GCOL + (s % DBLK) * BPG + s // DBLK


def _rho_glob():
    """global node m -> (is_A, row in htabA/htabB).

    hloc is group-major; the AllGather pieces are A = local groups 0-4,
    B = groups 5-9, so tab row = core*2500 + (g mod 5)*500 + perm(s)."""
    m = np.arange(N)
    c, l = m // NLOC, m % NLOC
    g, s = l // GCOL, l % GCOL
    is_A = g < NG_A
    perm = (s % DBLK) * BPG + s // DBLK
    row = np.where(
        is_A,
        c * (NG_A * GCOL) + g * GCOL + perm,
        c * ((NG - NG_A) * GCOL) + (g - NG_A) * GCOL + perm,
    )
    return is_A, row.astype(np.int64)


def _prepare(x, edge_index, W1l, b1l, W1r, b1r, W2l, b2l, W2r, b2r):
    src = np.asarray(edge_index[0], dtype=np.int64)
    dst = np.asarray(edge_index[1], dtype=np.int64)
    x = np.ascontiguousarray(np.asarray(x, dtype=np.float32))
    x_f8 = x.astype(ml_dtypes.float8_e4m3fn)

    cnt = np.bincount(dst, minlength=N).astype(np.float32)
    inv_cnt = 1.0 / np.maximum(cnt, 1.0)

    order = np.argsort(dst, kind="stable")
    src_sorted = src[order].astype(np.int64)
    dst_sorted = dst[order]

    rho_loc = _rho_loc()
    is_A_tab, rho_glob = _rho_glob()
    row2 = rho_glob[src_sorted]          # layer-2 tab row per sorted edge
    is_lo = is_A_tab[src_sorted]         # "lo" = htabA, "hi" = htabB

    # global tile boundaries (dst-sorted)
    tile_edges = np.searchsorted(dst_sorted, np.arange(0, N + 1, DT))
    counts = np.diff(tile_edges).reshape(NCORES, NT)
    kc1 = np.maximum(np.ceil(counts.max(axis=0) / P).astype(int), 1)
    kp1 = int(kc1.max())
    coff1 = np.concatenate([[0], np.cumsum(kc1)])
    nch1 = int(kc1.sum())

    # layer-2 lo/hi chunk counts per DT2 tile (max over cores)
    tile_edges2 = np.searchsorted(dst_sorted, np.arange(0, N + 1, DT2))
    counts2 = np.diff(tile_edges2).reshape(NCORES, NT2)
    nlo_ct = np.zeros((NCORES, NT2), np.int64)
    for c in range(NCORES):
        for t in range(NT2):
            gidx = c * NT2 + t
            nlo_ct[c, t] = int(
                is_lo[tile_edges2[gidx] : tile_edges2[gidx + 1]].sum()
            )
    nhi_ct = counts2 - nlo_ct
    klo = np.maximum(np.ceil(nlo_ct.max(axis=0) / P).astype(int), 1)
    khi = np.maximum(np.ceil(nhi_ct.max(axis=0) / P).astype(int), 1)
    kc2 = klo + khi
    kp2 = int(kc2.max())
    nlo = int(klo.sum())
    nhi = int(khi.sum())
    loff_lo = np.concatenate([[0], np.cumsum(klo)])
    loff_hi = np.concatenate([[0], np.cumsum(khi)])
    glo = np.concatenate([[0], np.cumsum(klo.reshape(NG, GT2).sum(axis=1))])
    ghi = np.concatenate([[0], np.cumsum(khi.reshape(NG, GT2).sum(axis=1))])

    meta = (
        tuple(int(v) for v in kc1), kp1,
        tuple(int(v) for v in klo), tuple(int(v) for v in khi), kp2,
    )

    def iota_arr(kp, dt):
        a = np.zeros((P, dt * kp), ml_dtypes.bfloat16)
        a[:, :] = np.repeat(np.arange(dt, dtype=np.float32), kp)[None, :]
        return a

    w1lT_np = np.ascontiguousarray(np.asarray(W1l, np.float32).T.astype(ml_dtypes.bfloat16))
    w1rT_np = np.ascontiguousarray(np.asarray(W1r, np.float32).T.astype(ml_dtypes.bfloat16))
    w2lT_np = np.ascontiguousarray(np.asarray(W2l, np.float32).T.astype(ml_dtypes.bfloat16))
    w2rT_np = np.ascontiguousarray(np.asarray(W2r, np.float32).T.astype(ml_dtypes.bfloat16))
    b1_np = np.asarray(b1l, np.float32) + np.asarray(b1r, np.float32)
    b2_np = np.asarray(b2l, np.float32) + np.asarray(b2r, np.float32)
    b1c_np = np.ascontiguousarray(b1_np[:, None])
    b1row_np = np.zeros((P, C), ml_dtypes.bfloat16)
    b1row_np[0, :] = b1_np.astype(ml_dtypes.bfloat16)
    b2row_np = np.zeros((P, O), ml_dtypes.bfloat16)
    b2row_np[0, :] = b2_np.astype(ml_dtypes.bfloat16)
    xT_full = np.ascontiguousarray(x.T.astype(ml_dtypes.bfloat16))
    iota1_np = iota_arr(kp1, DT)
    iota2_np = iota_arr(kp2, DT2)

    in_maps = []
    for c in range(NCORES):
        base = c * NLOC
        src1_cols = np.zeros((nch1, P), np.int64)    # layer-1 chunk src ids
        dval1_np = np.full((NT * kp1, P), -1.0, np.float32)
        dval2_np = np.full((NT2 * kp2, P), -1.0, np.float32)
        lo_rows = np.zeros((nlo, P), np.int16)
        hi_rows = np.zeros((max(nhi, 1), P), np.int16)
        for t in range(NT):
            gidx = c * NT + t
            e0, e1 = tile_edges[gidx], tile_edges[gidx + 1]
            s = src_sorted[e0:e1]
            d = (dst_sorted[e0:e1] - (base + t * DT)).astype(np.float32)
            o = np.argsort(s, kind="stable")
            s, d = s[o], d[o]

            # layer 1: all edges, packed to kc1[t] chunks
            k_t = int(kc1[t])
            fs = np.zeros(k_t * P, np.int64)
            fd = np.full(k_t * P, -1.0, np.float32)
            fs[: len(s)] = s
            fd[: len(s)] = d
            src1_cols[coff1[t] : coff1[t + 1]] = fs.reshape(k_t, P)
            dval1_np[t * kp1 : t * kp1 + k_t] = fd.reshape(k_t, P)

        for t in range(NT2):
            gidx = c * NT2 + t
            e0, e1 = tile_edges2[gidx], tile_edges2[gidx + 1]
            s = src_sorted[e0:e1]
            r2 = row2[e0:e1]
            a_m = is_lo[e0:e1]
            d = (dst_sorted[e0:e1] - (base + t * DT2)).astype(np.float32)
            o = np.argsort(s, kind="stable")
            d, r2, lo_m = d[o], r2[o], a_m[o]

            # layer 2: lo chunks then hi chunks
            k_l, k_h = int(klo[t]), int(khi[t])
            fr = np.zeros(k_l * P, np.int16)
            fd2 = np.full((k_l + k_h) * P, -1.0, np.float32)
            n_l = int(lo_m.sum())
            fr[:n_l] = r2[lo_m].astype(np.int16)
            fd2[:n_l] = d[lo_m]
            lo_rows[loff_lo[t] : loff_lo[t + 1]] = fr.reshape(k_l, P)
            if k_h > 0:
                frh = np.zeros(k_h * P, np.int16)
                n_h = int((~lo_m).sum())
                frh[:n_h] = r2[~lo_m].astype(np.int16)
                fd2[k_l * P : k_l * P + n_h] = d[~lo_m]
                hi_rows[loff_hi[t] : loff_hi[t + 1]] = frh.reshape(k_h, P)
            dval2_np[t * kp2 : t * kp2 + k_l + k_h] = fd2.reshape(k_l + k_h, P)

        # layer-1 message stream: x[src], chunk-major
        mstream_np = (
            x_f8[src1_cols].transpose(1, 0, 2).reshape(P, nch1 * C)
        )
        # dma_gather index arrays, wrapped per group
        idxlo_np = np.zeros((P, nlo * 8), np.int16)
        idxhi_np = np.zeros((P, max(nhi, 1) * 8), np.int16)
        for g in range(NG):
            blk = lo_rows[glo[g] : glo[g + 1]].reshape(-1)
            idxlo_np[:, glo[g] * 8 : glo[g + 1] * 8] = _wrap16(blk)
            if ghi[g + 1] > ghi[g]:
                blk = hi_rows[ghi[g] : ghi[g + 1]].reshape(-1)
                idxhi_np[:, ghi[g] * 8 : ghi[g + 1] * 8] = _wrap16(blk)

        in_maps.append(
            {
                "mstream": np.ascontiguousarray(mstream_np),
                "xT": np.ascontiguousarray(xT_full[:, base : base + NLOC]),
                "invc": np.broadcast_to(
                    inv_cnt[base : base + NLOC].astype(ml_dtypes.bfloat16),
                    (P, NLOC),
                ).copy(),
                "iota1": iota1_np,
                "iota2": iota2_np,
                "dval1": np.ascontiguousarray(
                    dval1_np.T.astype(ml_dtypes.bfloat16)
                ),
                "dval2": np.ascontiguousarray(
                    dval2_np.T.astype(ml_dtypes.bfloat16)
                ),
                "idxlo": idxlo_np,
                "idxhi": idxhi_np,
                "w1lT": w1lT_np,
                "w1rT": w1rT_np,
                "w2lT": w2lT_np,
                "w2rT": w2rT_np,
                "b1c": b1c_np,
                "b1row": b1row_np,
                "b2row": b2row_np,
            }
        )
    return meta, rho_loc, in_maps


def _install_profile_hook():
    """The stripped agent image lacks antenv.axon_hooks; synthesize it and
    register the ctypes NTFF profile hook so trace=True works."""
    import sys
    import types

    if "antenv.axon_hooks" in sys.modules:
        return
    import antenv

    mod = types.ModuleType("antenv.axon_hooks")
    state = {"hook": None}
    mod.set_axon_ntff_profile_hook = lambda h: state.update(hook=h)
    mod.get_axon_ntff_profile_hook = lambda: state["hook"]
    sys.modules["antenv.axon_hooks"] = mod
    antenv.axon_hooks = mod

    from trn_agent_boot.trn_boot import _ntff_profile_via_ctypes

    mod.set_axon_ntff_profile_hook(
        _ntff_profile_via_ctypes("/opt/axon/libaxon_pjrt.so")
    )

    import concourse.bass_utils as bu

    bu.upload_artifacts = lambda tmpdir: tmpdir  # no remote bucket here


def kernel(trace=False, **inputs):
    if trace:
        _install_profile_hook()
    meta, rho_loc, in_maps = _prepare(**inputs)
    nc = _build(meta)
    res = run_bass_kernel_spmd(nc, in_maps, list(range(NCORES)), trace=trace)
    out = np.concatenate(
        [res.results[c]["out"][rho_loc] for c in range(NCORES)], axis=0
    )
    if trace:
        return out, res
    return out


# revision 30
# speedup vs baseline: 1.0087x; 1.0087x over previous
"""GraphSAGE (2-layer, mean aggregation) on 8 Trainium2 NeuronCores.

Sharding: nodes partitioned by dst range across 8 cores (graph parallel).
Layer-1 edge messages x[src] are a compile-time permutation, so the host
stages them as a contiguous fp8e4m3 stream (direct DMA at full bandwidth;
the fp8 quantization adds ~3e-3 rms).  Layer-2 messages h[src] are batch-
gathered with SWDGE dma_gather (1024 indices per call — the descriptor
ring's hard limit — on 4 rotating queues) from two AllGathered node-major
h tables: piece A holds every core's L1 groups 0-4 and is AllGathered
mid-L1 (hidden under compute), piece B (groups 5-9) right after L1.  Both
tables are under 32768 rows, so int16 gather indices need no rebasing;
each edge is classed A/B by its source's L1 group.  Gathers wait for a
1-row barrier AllGather after piece B — collective completion alone races
remote stripe visibility (seen as rare NaN/garbage rows) — and are pinned
behind the collective dispatches on the Pool FIFO with no-sync deps, since
overlapping SWDGE gathers starve the collective's DMA rings ~2.5x.
Segment-sum runs as TensorE matmuls against one-hot dst-selection tiles
generated on-device by one DVE iota==dst compare per group (fp8 for L1 to
match the message dtype).  Dense SAGE transforms emit node-major blocks
directly (PE contracts channels with the node-block as the stationary
operand; bias via a rank-1 e0 x b matmul), so no transposes are needed.
Node rows use a group-major permuted layout (local node 500g+125b+p at row
500g+4p+b) so every hloc/out DMA write is contiguous; the host un-permutes.
Weights are replicated; PSUM accumulates in f32; output is f32.
"""

import ml_dtypes
import numpy as np

import concourse.bass as bass
import concourse.library_config as library_config
import concourse.mybir as mybir
import concourse.tile as tile
from concourse.bass_utils import run_bass_kernel_spmd
from concourse.library_overlay import lower_extended_insts
from concourse.tile import ScopedClock

# ---------------------------------------------------------------------------
# Workarounds for this container's walrus codegen: instructions can carry at
# most one sync-wait command ("Too many sync wait commands" otherwise), and
# Drain-based barriers reject waits entirely.
# ---------------------------------------------------------------------------


def _drain_and_barrier(self, tick_clock, wait_clock):
    nop_inst = self.nc.sync.nop(nofuse=True, hint="pre_drain_waits")
    wait_clock.add_sem_waits(
        nop_inst.ins, ScopedClock({None: tick_clock.global_clock})
    )
    si = nop_inst.ins.sync_info
    waits = list(si.on_wait) if si and si.on_wait else []
    if len(waits) > 1:
        si.on_wait = waits[:1]
        for w in waits[1:]:
            extra = self.nc.sync.nop(nofuse=True, hint="pre_drain_waits_x")
            extra.ins.sync_info = type(si)(on_wait=[w], on_update=[])
    self.nc.sync.drain()
    self.nc.all_engine_barrier(sem_only=True)
    assert self.sems is not None
    popped = self.nc._tile_sem_poison_stack.pop()
    assert popped is self._sem_poison
    self.nc.clear_and_free_semaphores(list(self.sems.allocated().values()))
    self.nc.all_engine_barrier(sem_only=True)


tile.TileContext._drain_and_barrier = _drain_and_barrier


def _split_multi_waits(nc, maxw=1):
    """Move excess sync-waits onto same-engine NOPs inserted before."""
    n = 0
    for blk in nc.m.functions[0].blocks:
        il = blk.instructions
        i = 0
        while i < len(il):
            inst = il[i]
            si = inst.sync_info
            waits = list(si.on_wait) if si and si.on_wait else []
            if len(waits) > maxw:
                si.on_wait = waits[-maxw:]
                for w in waits[:-maxw]:
                    nop = mybir.InstNoOp(
                        name=f"wsplit-{n}",
                        engine=inst.engine,
                        sync_info=mybir.SyncInfo(on_wait=[w], on_update=[]),
                    )
                    n += 1
                    il.insert(i, nop)
                    i += 1
            i += 1


# ---------------------------------------------------------------------------

N = 40000
E = 640000
C = 128          # in/hidden channels
O = 121          # out channels
NCORES = 8
NLOC = N // NCORES       # 5000 dst nodes per core
DT = 50                  # dst nodes per aggregation tile
NT = NLOC // DT          # 100 dst tiles per core
GT = 10                  # tiles per pipeline group
NG = NT // GT            # 10 groups per layer
GCOL = GT * DT           # 500 agg columns per group
DBLK = 125               # nodes per dense output block
NBLK = NLOC // DBLK      # 40 dense blocks
BPG = GCOL // DBLK       # 4 dense blocks per group
P = 128                  # edges per chunk (matmul contraction dim)
DT2 = 100                # layer-2 dst tile (coarser: less ceil padding)
NT2 = NLOC // DT2        # 50 layer-2 tiles
GT2 = GCOL // DT2        # 5 layer-2 tiles per group
GBUFS = 4                # L2 gathered-message buffers in flight
NG_A = 5                 # L1 groups in AllGather piece A (piece B = rest)
F32 = mybir.dt.float32
BF16 = mybir.dt.bfloat16
FP8 = mybir.dt.float8e4
I16 = mybir.dt.int16

_cache = {}
PHASE = 5  # 1=L1, 2=+AllGather, 3=+L2 gathers, 4=+L2 agg, 5=full


def _build(meta):
    """meta = (kc1, kp1, klo, khi, kp2) chunk structure, SPMD-identical."""
    key = (meta, PHASE)
    if key in _cache:
        return _cache[key]
    kc1, kp1, klo, khi, kp2 = meta   # klo/khi = A/B half chunk counts
    kc1 = np.array(kc1)
    klo = np.array(klo)
    khi = np.array(khi)
    kc2 = klo + khi
    coff1 = np.concatenate([[0], np.cumsum(kc1)])
    nch1 = int(coff1[-1])
    nlo = int(klo.sum())
    nhi = int(khi.sum())
    # per-group A/B chunk offsets for layer 2
    glo = np.concatenate([[0], np.cumsum(klo.reshape(NG, GT2).sum(axis=1))])
    ghi = np.concatenate([[0], np.cumsum(khi.reshape(NG, GT2).sum(axis=1))])

    nc = bass.Bass(num_swdge_queues=4)
    mstream = nc.dram_tensor("mstream", [P, nch1 * C], FP8, kind="ExternalInput")
    xT = nc.dram_tensor("xT", [C, NLOC], BF16, kind="ExternalInput")
    invc = nc.dram_tensor("invc", [P, NLOC], BF16, kind="ExternalInput")
    iota1 = nc.dram_tensor("iota1", [P, DT * kp1], BF16, kind="ExternalInput")
    iota2 = nc.dram_tensor("iota2", [P, DT2 * kp2], BF16, kind="ExternalInput")
    dval1 = nc.dram_tensor("dval1", [P, NT * kp1], BF16, kind="ExternalInput")
    dval2 = nc.dram_tensor("dval2", [P, NT2 * kp2], BF16, kind="ExternalInput")
    idxlo = nc.dram_tensor("idxlo", [P, nlo * 8], I16, kind="ExternalInput")
    idxhi = nc.dram_tensor("idxhi", [P, max(nhi, 1) * 8], I16, kind="ExternalInput")
    w1lT = nc.dram_tensor("w1lT", [C, C], BF16, kind="ExternalInput")
    w1rT = nc.dram_tensor("w1rT", [C, C], BF16, kind="ExternalInput")
    w2lT = nc.dram_tensor("w2lT", [C, O], BF16, kind="ExternalInput")
    w2rT = nc.dram_tensor("w2rT", [C, O], BF16, kind="ExternalInput")
    b1c = nc.dram_tensor("b1c", [C, 1], F32, kind="ExternalInput")
    b1row = nc.dram_tensor("b1row", [P, C], BF16, kind="ExternalInput")
    b2row = nc.dram_tensor("b2row", [P, O], BF16, kind="ExternalInput")
    out = nc.dram_tensor("out", [NLOC, O], F32, kind="ExternalOutput")

    with tile.TileContext(nc) as tc:
        with (
            tc.tile_pool(name="const", bufs=1) as cpool,
            tc.tile_pool(name="feat", bufs=1) as fpool,
            tc.tile_pool(name="slab", bufs=3) as spool1,
            tc.tile_pool(name="gbig", bufs=GBUFS) as gpool,
            tc.tile_pool(name="agg", bufs=2) as apool,
            tc.tile_pool(name="oh", bufs=2) as opool,
            tc.tile_pool(name="stg", bufs=2) as spool,
            tc.tile_pool(name="psum_a", bufs=2, space="PSUM") as pa,
            tc.tile_pool(name="psum_d", bufs=2, space="PSUM") as pd,
            tc.tile_pool(name="psum_n", bufs=3, space="PSUM") as pn,
            tc.tile_pool(name="dram", bufs=1, space="DRAM") as dpool,
        ):
            # ---- resident tiles -------------------------------------------
            xT_s = fpool.tile([C, NLOC], BF16)
            invc_s = fpool.tile([P, NLOC], BF16)
            iota1_s = fpool.tile([P, DT * kp1], BF16)
            iota2_s = fpool.tile([P, DT2 * kp2], BF16)
            dval1_s = fpool.tile([P, NT * kp1], BF16)
            dval2_s = fpool.tile([P, NT2 * kp2], BF16)
            idxlo_s = fpool.tile([P, nlo * 8], I16)
            idxhi_s = fpool.tile([P, max(nhi, 1) * 8], I16)
            hT_s = fpool.tile([C, NLOC], BF16)
            w1lT_s = cpool.tile([C, C], BF16)
            w1rT_s = cpool.tile([C, C], BF16)
            w2lT_s = cpool.tile([C, O], BF16)
            w2rT_s = cpool.tile([C, O], BF16)
            b1c_s = cpool.tile([C, 1], F32)
            b1row_s = cpool.tile([P, C], BF16)
            b2row_s = cpool.tile([P, O], BF16)
            e0_s = cpool.tile([P, DBLK], BF16)
            zer_s = cpool.tile([P, 1], F32)

            hloc = dpool.tile([NLOC, C], BF16)
            htabA = dpool.tile([NCORES * NG_A * GCOL, C], BF16, addr_space="Shared")
            htabB = dpool.tile([NCORES * (NG - NG_A) * GCOL, C], BF16, addr_space="Shared")
            agbar = dpool.tile([NCORES, C], BF16, addr_space="Shared")

            nc.sync.dma_start(out=iota1_s[:], in_=iota1[:])
            nc.sync.dma_start(out=dval1_s[:], in_=dval1[:])
            nc.sync.dma_start(out=w1lT_s[:], in_=w1lT[:])
            nc.sync.dma_start(out=w1rT_s[:], in_=w1rT[:])
            nc.sync.dma_start(out=b1c_s[:], in_=b1c[:])
            nc.sync.dma_start(out=b1row_s[:], in_=b1row[:])
            nc.gpsimd.memset(e0_s[:], 0.0)
            nc.gpsimd.memset(e0_s[0:1, :], 1.0)
            nc.gpsimd.memset(zer_s[:], 0.0)
            nc.gpsimd.load_library(library_config.mlp)

            # node rows permuted so every hloc/out DMA write is contiguous:
            # local node n = 500g + 125b + p  <->  row 500g + 4p + b
            ag_insts = []
            hloc_v = hloc[:, :].rearrange("(g p b) c -> p g b c", p=DBLK, b=BPG)
            out_v = out[:, :].rearrange("(g p b) o -> p g b o", p=DBLK, b=BPG)

            def onehot(g, iota_s, dval_s, kp, dt, gt, dtype):
                """One DVE is_equal builds the group's one-hot block:
                oh[p, t*dt*kp + d*kp + j] = (dst_off(tile t, chunk j, lane p) == d)."""
                oh = opool.tile([P, gt * dt * kp], dtype, tag="oh")
                oh4 = oh[:, :].rearrange(
                    "p (t d j) -> p t d j", t=gt, d=dt, j=kp
                )
                iota4 = (
                    iota_s[:, :]
                    .rearrange("p (d j) -> p d j", j=kp)
                    .unsqueeze(1)
                    .to_broadcast((P, gt, dt, kp))
                )
                dval4 = (
                    dval_s[:, gt * kp * g : gt * kp * (g + 1)]
                    .rearrange("p (t j) -> p t j", j=kp)
                    .unsqueeze(2)
                    .to_broadcast((P, gt, dt, kp))
                )
                nc.vector.tensor_tensor(
                    out=oh4, in0=iota4, in1=dval4, op=mybir.AluOpType.is_equal
                )
                return oh4

            def aggregate(g, oh4, lhs, kcs, dt, gt):
                """Accumulate the group's [C, 500] segment-sum and normalize."""
                ps = pa.tile([C, GCOL], F32, space="PSUM")
                for t in range(gt):
                    tt = gt * g + t
                    k_t = int(kcs[tt])
                    for k in range(k_t):
                        nc.tensor.matmul(
                            out=ps[:, t * dt : (t + 1) * dt],
                            lhsT=lhs(tt, k),
                            rhs=oh4[:, t, :, k],
                            start=(k == 0),
                            stop=(k == k_t - 1),
                        )
                agg = apool.tile([C, GCOL], BF16, tag="agg")
                cols = slice(g * GCOL, (g + 1) * GCOL)
                nc.scalar.activation(
                    agg[:], ps[:], mybir.ActivationFunctionType.Copy
                )
                nc.vector.tensor_mul(
                    out=agg[:], in0=agg[:], in1=invc_s[:, cols]
                )
                return agg

            # ---- layer 1 ---------------------------------------------------
            for g in range(NG):
                j0, j1 = int(coff1[GT * g]), int(coff1[GT * (g + 1)])
                kg = j1 - j0
                slab = spool1.tile([P, kg * C], FP8, tag="slab")
                nc.sync.dma_start(
                    out=slab[:], in_=mstream[:, j0 * C : j1 * C]
                )
                if g == 0:
                    # after group 0's slab so that DMA issues first, but
                    # before any consumer of xT/invc in program order
                    # scalar HWDGE ring: parallel with slab prefetch
                    nc.scalar.dma_start(out=xT_s[:], in_=xT[:])
                    nc.scalar.dma_start(out=invc_s[:], in_=invc[:])
                if g == 1:
                    # L2-only inputs: off the L1-critical DMA prefix
                    nc.scalar.dma_start(out=iota2_s[:], in_=iota2[:])
                    nc.scalar.dma_start(out=dval2_s[:], in_=dval2[:])
                    nc.scalar.dma_start(out=idxlo_s[:], in_=idxlo[:])
                    nc.scalar.dma_start(out=idxhi_s[:], in_=idxhi[:])
                    nc.scalar.dma_start(out=w2lT_s[:], in_=w2lT[:])
                    nc.scalar.dma_start(out=w2rT_s[:], in_=w2rT[:])
                    nc.scalar.dma_start(out=b2row_s[:], in_=b2row[:])
                oh4 = onehot(g, iota1_s, dval1_s, kp1, DT, GT, FP8)
                agg = aggregate(
                    g, oh4,
                    lambda tt, k: slab[
                        :, (int(coff1[tt]) - j0 + k) * C
                        : (int(coff1[tt]) - j0 + k + 1) * C
                    ],
                    kc1, DT, GT,
                )
                cols = slice(g * GCOL, (g + 1) * GCOL)
                # channel-major dense -> hT
                ph = pd.tile([C, GCOL], F32, space="PSUM")
                nc.tensor.matmul(
                    out=ph[:], lhsT=w1lT_s[:], rhs=agg[:],
                    start=True, stop=False,
                )
                nc.tensor.matmul(
                    out=ph[:], lhsT=w1rT_s[:], rhs=xT_s[:, cols],
                    start=False, stop=True,
                )
                nc.scalar.activation(
                    hT_s[:, cols], ph[:],
                    mybir.ActivationFunctionType.Relu, bias=b1c_s[:, :1],
                )
                # node-major dense -> h blocks -> hloc (permuted rows)
                hstg = spool.tile([DBLK, BPG * C], BF16, tag="hstg")
                for b in range(BPG):
                    nb = slice((g * BPG + b) * DBLK, (g * BPG + b + 1) * DBLK)
                    pnb = pn.tile([DBLK, C], F32, space="PSUM")
                    nc.tensor.matmul(
                        out=pnb[:], lhsT=agg[:, b * DBLK : (b + 1) * DBLK],
                        rhs=w1lT_s[:],
                        start=True, stop=False,
                    )
                    nc.tensor.matmul(
                        out=pnb[:], lhsT=xT_s[:, nb], rhs=w1rT_s[:],
                        start=False, stop=False,
                    )
                    nc.tensor.matmul(
                        out=pnb[:], lhsT=e0_s[:], rhs=b1row_s[:],
                        start=False, stop=True,
                    )
                    nc.scalar.activation(
                        hstg[:, b * C : (b + 1) * C], pnb[:],
                        mybir.ActivationFunctionType.Relu,
                        bias=zer_s[:DBLK, :1],
                    )
                nc.sync.dma_start(
                    out=hloc_v[:, g, :, :],
                    in_=hstg[:, :].rearrange("p (b c) -> p b c", c=C),
                )
                # two-piece AllGather: A (groups 0-4) overlaps L1's tail,
                # B (groups 5-9) right after it; Shared outputs take the
                # fast collective path (one writer per tensor)
                if PHASE >= 2 and g in (NG_A - 1, NG - 1):
                    piece = htabA if g < NG_A else htabB
                    lo = 0 if g < NG_A else NG_A * GCOL
                    ag_insts.append(nc.gpsimd.collective_compute(
                        "AllGather",
                        mybir.AluOpType.bypass,
                        replica_groups=[list(range(NCORES))],
                        ins=[
                            hloc[
                                lo : lo
                                + (NG_A if g < NG_A else NG - NG_A) * GCOL,
                                :,
                            ].opt()
                        ],
                        outs=[piece[:, :].opt()],
                    ))
                    if g == NG - 1:
                        # 1-row barrier AllGather: completion implies every
                        # peer's AG-B stripes drained into our htab (same
                        # CC queue + link FIFO), closing the visibility race
                        ag_insts.append(nc.gpsimd.collective_compute(
                            "AllGather",
                            mybir.AluOpType.bypass,
                            replica_groups=[list(range(NCORES))],
                            ins=[hloc[0:1, :].opt()],
                            outs=[agbar[:, :].opt()],
                        ))

            # ---- layer 2 ---------------------------------------------------
            loff_lo = np.concatenate([[0], np.cumsum(klo)])
            loff_hi = np.concatenate([[0], np.cumsum(khi)])
            _nreg = {}

            def nreg(v):
                if v not in _nreg:
                    _nreg[v] = nc.gpsimd.to_reg(v)
                return _nreg[v]

            bigs = {}
            qrr = [0]
            GMAX = 8

            first_cls = {}

            def gather_half(big, kgh, base_col, idx_s, goff, tab, cls):
                for c0 in range(0, kgh, GMAX):
                    ncall = min(GMAX, kgh - c0)
                    col = base_col + c0
                    inst = nc.gpsimd.dma_gather(
                        out_ap=big[
                            :, col * C : (col + ncall) * C
                        ].rearrange("p (k c) -> p k c", c=C),
                        in_ap=tab,
                        idxs_ap=idx_s[
                            :, (goff + c0) * 8 : (goff + c0 + ncall) * 8
                        ],
                        num_idxs=ncall * P,
                        num_idxs_reg=nreg(ncall * P),
                        elem_size=C,
                        queue_num=qrr[0] % nc.num_swdge_queues,
                    )
                    # keep gathers behind the collective dispatches on the
                    # Pool FIFO (Tile would otherwise hoist them, starving
                    # the collective's DMA rings); the first gather of each
                    # class waits for the NEXT collective after its table to
                    # COMPLETE — one-collective-later completion implies the
                    # peers' stripes for this table drained (link FIFO),
                    # closing the remote-visibility race
                    deps = bass.InstructionNameOrderedSet()
                    for a in ag_insts:
                        deps.add(a.ins.name)
                    inst.ins.add_nosync_dependencies_from(deps)
                    if cls not in first_cls:
                        first_cls[cls] = inst
                        gate = ag_insts[1] if cls == "A" else ag_insts[2]
                        bass._add_dep_helper(
                            inst.ins, gate.ins, sync=True,
                            reason=f"class-{cls} gathers after settle",
                        )
                    qrr[0] += 1

            hi_pending = []

            def prep(g, hi_now=True):
                """Issue group g's gather calls (A half, then B half)."""
                kglo = int(glo[g + 1] - glo[g])
                kghi = int(ghi[g + 1] - ghi[g])
                big = gpool.tile([P, (kglo + kghi) * C], BF16, tag="gbig")
                gather_half(
                    big, kglo, 0, idxlo_s, int(glo[g]), htabA[:, :], "A"
                )
                if hi_now:
                    gather_half(
                        big, kghi, kglo, idxhi_s, int(ghi[g]),
                        htabB[:, :], "B",
                    )
                else:
                    hi_pending.append((big, kghi, kglo, int(ghi[g])))
                bigs[g] = big

            def process(g):
                kglo = int(glo[g + 1] - glo[g])
                big = bigs.pop(g)
                if PHASE < 4:
                    if g + GBUFS < NG:
                        prep(g + GBUFS)
                    return

                def lhs2(tt, k, g=g, big=big, kglo=kglo):
                    if k < klo[tt]:
                        jc = int(loff_lo[tt] - glo[g]) + k
                    else:
                        jc = (
                            kglo
                            + int(loff_hi[tt] - ghi[g])
                            + (k - int(klo[tt]))
                        )
                    return big[:, jc * C : (jc + 1) * C]

                oh4 = onehot(g, iota2_s, dval2_s, kp2, DT2, GT2, BF16)
                agg = aggregate(g, oh4, lhs2, kc2, DT2, GT2)
                if PHASE < 5:
                    if g + GBUFS < NG:
                        prep(g + GBUFS)
                    return
                # node-major dense -> out blocks (permuted rows)
                ostg = spool.tile([DBLK, BPG * O], F32, tag="ostg")
                for b in range(BPG):
                    nb = slice((g * BPG + b) * DBLK, (g * BPG + b + 1) * DBLK)
                    pnb = pn.tile([DBLK, O], F32, space="PSUM")
                    nc.tensor.matmul(
                        out=pnb[:], lhsT=agg[:, b * DBLK : (b + 1) * DBLK],
                        rhs=w2lT_s[:],
                        start=True, stop=False,
                    )
                    nc.tensor.matmul(
                        out=pnb[:], lhsT=hT_s[:, nb], rhs=w2rT_s[:],
                        start=False, stop=False,
                    )
                    nc.tensor.matmul(
                        out=pnb[:], lhsT=e0_s[:], rhs=b2row_s[:],
                        start=False, stop=True,
                    )
                    nc.scalar.activation(
                        ostg[:, b * O : (b + 1) * O], pnb[:],
                        mybir.ActivationFunctionType.Copy,
                    )
                nc.sync.dma_start(
                    out=out_v[:, g, :, :],
                    in_=ostg[:, :].rearrange("p (b o) -> p b o", o=O),
                )
                if g + GBUFS < NG:
                    prep(g + GBUFS)

            if PHASE >= 3:
                # A-half gathers of the first GBUFS groups run right after
                # AllGather-A (overlapping L1's tail and AllGather-B); their
                # B halves follow so the Pool FIFO never stalls on piece B
                # A-halves of the first GBUFS groups dispatch right after
                # AllGather-B completes, overlapping the barrier's settle
                # window; their B-halves follow so the Pool FIFO never
                # stalls on the barrier
                for g in range(GBUFS):
                    prep(g, hi_now=False)
                for big, kghi, kglo, goff in hi_pending:
                    gather_half(big, kghi, kglo, idxhi_s, goff,
                                htabB[:, :], "B")
                for g in range(NG):
                    process(g)

    _split_multi_waits(nc)
    lower_extended_insts(nc)
    _cache[key] = nc
    return nc


def _wrap16(vals):
    """dma_gather index layout: linear idx i at partition i%16, col i//16,
    replicated across the 8 Q7-core partition stripes."""
    v = np.asarray(vals, np.int16).reshape(-1, 16)
    return np.tile(v.T, (P // 16, 1))


def _rho_loc():
    """local node n = 500g + 125b + p  ->  hloc/out row 500g + 4p + b"""
    n = np.arange(NLOC)
    g, s = n // GCOL, n % GCOL
    return g * GCOL + (s % DBLK) * BPG + s // DBLK


def _rho_glob():
    """global node m -> (is_A, row in htabA/htabB).

    hloc is group-major; the AllGather pieces are A = local groups 0-4,
    B = groups 5-9, so tab row = core*2500 + (g mod 5)*500 + perm(s)."""
    m = np.arange(N)
    c, l = m // NLOC, m % NLOC
    g, s = l // GCOL, l % GCOL
    is_A = g < NG_A
    perm = (s % DBLK) * BPG + s // DBLK
    row = np.where(
        is_A,
        c * (NG_A * GCOL) + g * GCOL + perm,
        c * ((NG - NG_A) * GCOL) + (g - NG_A) * GCOL + perm,
    )
    return is_A, row.astype(np.int64)


def _prepare(x, edge_index, W1l, b1l, W1r, b1r, W2l, b2l, W2r, b2r):
    src = np.asarray(edge_index[0], dtype=np.int64)
    dst = np.asarray(edge_index[1], dtype=np.int64)
    x = np.ascontiguousarray(np.asarray(x, dtype=np.float32))
    x_f8 = x.astype(ml_dtypes.float8_e4m3fn)

    cnt = np.bincount(dst, minlength=N).astype(np.float32)
    inv_cnt = 1.0 / np.maximum(cnt, 1.0)

    order = np.argsort(dst, kind="stable")
    src_sorted = src[order].astype(np.int64)
    dst_sorted = dst[order]

    rho_loc = _rho_loc()
    is_A_tab, rho_glob = _rho_glob()
    row2 = rho_glob[src_sorted]          # layer-2 tab row per sorted edge
    is_lo = is_A_tab[src_sorted]         # "lo" = htabA, "hi" = htabB

    # global tile boundaries (dst-sorted)
    tile_edges = np.searchsorted(dst_sorted, np.arange(0, N + 1, DT))
    counts = np.diff(tile_edges).reshape(NCORES, NT)
    kc1 = np.maximum(np.ceil(counts.max(axis=0) / P).astype(int), 1)
    kp1 = int(kc1.max())
    coff1 = np.concatenate([[0], np.cumsum(kc1)])
    nch1 = int(kc1.sum())

    # layer-2 lo/hi chunk counts per DT2 tile (max over cores)
    tile_edges2 = np.searchsorted(dst_sorted, np.arange(0, N + 1, DT2))
    counts2 = np.diff(tile_edges2).reshape(NCORES, NT2)
    nlo_ct = np.zeros((NCORES, NT2), np.int64)
    for c in range(NCORES):
        for t in range(NT2):
            gidx = c * NT2 + t
            nlo_ct[c, t] = int(
                is_lo[tile_edges2[gidx] : tile_edges2[gidx + 1]].sum()
            )
    nhi_ct = counts2 - nlo_ct
    klo = np.maximum(np.ceil(nlo_ct.max(axis=0) / P).astype(int), 1)
    khi = np.maximum(np.ceil(nhi_ct.max(axis=0) / P).astype(int), 1)
    kc2 = klo + khi
    kp2 = int(kc2.max())
    nlo = int(klo.sum())
    nhi = int(khi.sum())
    loff_lo = np.concatenate([[0], np.cumsum(klo)])
    loff_hi = np.concatenate([[0], np.cumsum(khi)])
    glo = np.concatenate([[0], np.cumsum(klo.reshape(NG, GT2).sum(axis=1))])
    ghi = np.concatenate([[0], np.cumsum(khi.reshape(NG, GT2).sum(axis=1))])

    meta = (
        tuple(int(v) for v in kc1), kp1,
        tuple(int(v) for v in klo), tuple(int(v) for v in khi), kp2,
    )

    def iota_arr(kp, dt):
        a = np.zeros((P, dt * kp), ml_dtypes.bfloat16)
        a[:, :] = np.repeat(np.arange(dt, dtype=np.float32), kp)[None, :]
        return a

    w1lT_np = np.ascontiguousarray(np.asarray(W1l, np.float32).T.astype(ml_dtypes.bfloat16))
    w1rT_np = np.ascontiguousarray(np.asarray(W1r, np.float32).T.astype(ml_dtypes.bfloat16))
    w2lT_np = np.ascontiguousarray(np.asarray(W2l, np.float32).T.astype(ml_dtypes.bfloat16))
    w2rT_np = np.ascontiguousarray(np.asarray(W2r, np.float32).T.astype(ml_dtypes.bfloat16))
    b1_np = np.asarray(b1l, np.float32) + np.asarray(b1r, np.float32)
    b2_np = np.asarray(b2l, np.float32) + np.asarray(b2r, np.float32)
    b1c_np = np.ascontiguousarray(b1_np[:, None])
    b1row_np = np.zeros((P, C), ml_dtypes.bfloat16)
    b1row_np[0, :] = b1_np.astype(ml_dtypes.bfloat16)
    b2row_np = np.zeros((P, O), ml_dtypes.bfloat16)
    b2row_np[0, :] = b2_np.astype(ml_dtypes.bfloat16)
    xT_full = np.ascontiguousarray(x.T.astype(ml_dtypes.bfloat16))
    iota1_np = iota_arr(kp1, DT)
    iota2_np = iota_arr(kp2, DT2)

    in_maps = []
    for c in range(NCORES):
        base = c * NLOC
        src1_cols = np.zeros((nch1, P), np.int64)    # layer-1 chunk src ids
        dval1_np = np.full((NT * kp1, P), -1.0, np.float32)
        dval2_np = np.full((NT2 * kp2, P), -1.0, np.float32)
        lo_rows = np.zeros((nlo, P), np.int16)
        hi_rows = np.zeros((max(nhi, 1), P), np.int16)
        for t in range(NT):
            gidx = c * NT + t
            e0, e1 = tile_edges[gidx], tile_edges[gidx + 1]
            s = src_sorted[e0:e1]
            d = (dst_sorted[e0:e1] - (base + t * DT)).astype(np.float32)
            o = np.argsort(s, kind="stable")
            s, d = s[o], d[o]

            # layer 1: all edges, packed to kc1[t] chunks
            k_t = int(kc1[t])
            fs = np.zeros(k_t * P, np.int64)
            fd = np.full(k_t * P, -1.0, np.float32)
            fs[: len(s)] = s
            fd[: len(s)] = d
            src1_cols[coff1[t] : coff1[t + 1]] = fs.reshape(k_t, P)
            dval1_np[t * kp1 : t * kp1 + k_t] = fd.reshape(k_t, P)

        for t in range(NT2):
            gidx = c * NT2 + t
            e0, e1 = tile_edges2[gidx], tile_edges2[gidx + 1]
            s = src_sorted[e0:e1]
            r2 = row2[e0:e1]
            a_m = is_lo[e0:e1]
            d = (dst_sorted[e0:e1] - (base + t * DT2)).astype(np.float32)
            o = np.argsort(s, kind="stable")
            d, r2, lo_m = d[o], r2[o], a_m[o]

            # layer 2: lo chunks then hi chunks
            k_l, k_h = int(klo[t]), int(khi[t])
            fr = np.zeros(k_l * P, np.int16)
            fd2 = np.full((k_l + k_h) * P, -1.0, np.float32)
            n_l = int(lo_m.sum())
            fr[:n_l] = r2[lo_m].astype(np.int16)
            fd2[:n_l] = d[lo_m]
            lo_rows[loff_lo[t] : loff_lo[t + 1]] = fr.reshape(k_l, P)
            if k_h > 0:
                frh = np.zeros(k_h * P, np.int16)
                n_h = int((~lo_m).sum())
                frh[:n_h] = r2[~lo_m].astype(np.int16)
                fd2[k_l * P : k_l * P + n_h] = d[~lo_m]
                hi_rows[loff_hi[t] : loff_hi[t + 1]] = frh.reshape(k_h, P)
            dval2_np[t * kp2 : t * kp2 + k_l + k_h] = fd2.reshape(k_l + k_h, P)

        # layer-1 message stream: x[src], chunk-major
        mstream_np = (
            x_f8[src1_cols].transpose(1, 0, 2).reshape(P, nch1 * C)
        )
        # dma_gather index arrays, wrapped per group
        idxlo_np = np.zeros((P, nlo * 8), np.int16)
        idxhi_np = np.zeros((P, max(nhi, 1) * 8), np.int16)
        for g in range(NG):
            blk = lo_rows[glo[g] : glo[g + 1]].reshape(-1)
            idxlo_np[:, glo[g] * 8 : glo[g + 1] * 8] = _wrap16(blk)
            if ghi[g + 1] > ghi[g]:
                blk = hi_rows[ghi[g] : ghi[g + 1]].reshape(-1)
                idxhi_np[:, ghi[g] * 8 : ghi[g + 1] * 8] = _wrap16(blk)

        in_maps.append(
            {
                "mstream": np.ascontiguousarray(mstream_np),
                "xT": np.ascontiguousarray(xT_full[:, base : base + NLOC]),
                "invc": np.broadcast_to(
                    inv_cnt[base : base + NLOC].astype(ml_dtypes.bfloat16),
                    (P, NLOC),
                ).copy(),
                "iota1": iota1_np,
                "iota2": iota2_np,
                "dval1": np.ascontiguousarray(
                    dval1_np.T.astype(ml_dtypes.bfloat16)
                ),
                "dval2": np.ascontiguousarray(
                    dval2_np.T.astype(ml_dtypes.bfloat16)
                ),
                "idxlo": idxlo_np,
                "idxhi": idxhi_np,
                "w1lT": w1lT_np,
                "w1rT": w1rT_np,
                "w2lT": w2lT_np,
                "w2rT": w2rT_np,
                "b1c": b1c_np,
                "b1row": b1row_np,
                "b2row": b2row_np,
            }
        )
    return meta, rho_loc, in_maps


def _install_profile_hook():
    """The stripped agent image lacks antenv.axon_hooks; synthesize it and
    register the ctypes NTFF profile hook so trace=True works."""
    import sys
    import types

    if "antenv.axon_hooks" in sys.modules:
        return
    import antenv

    mod = types.ModuleType("antenv.axon_hooks")
    state = {"hook": None}
    mod.set_axon_ntff_profile_hook = lambda h: state.update(hook=h)
    mod.get_axon_ntff_profile_hook = lambda: state["hook"]
    sys.modules["antenv.axon_hooks"] = mod
    antenv.axon_hooks = mod

    from trn_agent_boot.trn_boot import _ntff_profile_via_ctypes

    mod.set_axon_ntff_profile_hook(
        _ntff_profile_via_ctypes("/opt/axon/libaxon_pjrt.so")
    )

    import concourse.bass_utils as bu

    bu.upload_artifacts = lambda tmpdir: tmpdir  # no remote bucket here


def kernel(trace=False, **inputs):
    if trace:
        _install_profile_hook()
    meta, rho_loc, in_maps = _prepare(**inputs)
    nc = _build(meta)
    res = run_bass_kernel_spmd(nc, in_maps, list(range(NCORES)), trace=trace)
    out = np.concatenate(
        [res.results[c]["out"][rho_loc] for c in range(NCORES)], axis=0
    )
    if trace:
        return out, res
    return out
